# revision 1
# baseline (speedup 1.0000x reference)
"""Trainium2 Bass kernel for nn_Convolution (e3nn-style GNN message passing).

Strategy (8 NeuronCores, SPMD, no collectives):
- Sort edges by destination; core c owns destination nodes [6400c, 6400(c+1)).
- Per core: edges are binned into 50 node-blocks (128 nodes each) and padded to
  NG groups of 128 edges per block. Dummy edges gather a zero table row, so
  every tensor-product output term (all bilinear in source features) is 0.
- Gather source features with dma_gather from a 256B-padded table, split into
  lo/hi halves (int16 index limit), summed on DVE.
- Radial MLP layer 1 on PE with tile_position row-packed K=8 matmuls,
  layer 2 per-group with h as the stationary operand (w lands [edge, 256]).
- TP products on DVE via broadcast APs; the per-edge contraction over u is
  DEFERRED into the scatter matmul: one-hot(dst) x [512-wide product tile]
  accumulates in PSUM over each block, reduced over u once per block.
"""

import math
import os
import numpy as np

_TRACE_SIM = bool(int(os.environ.get('K_TRACE_SIM', '0')))
_NO_GATHER = bool(int(os.environ.get('K_NO_GATHER', '0')))
_NO_TP = bool(int(os.environ.get('K_NO_TP', '0')))
_NO_MM = bool(int(os.environ.get('K_NO_MM', '0')))


import concourse.bass as bass
import concourse.bacc as bacc
import concourse.mybir as mybir
from concourse.tile import TileContext
from concourse.bass_utils import run_bass_kernel_spmd

# ---------------- problem constants (hardcoded per spec) ----------------
N_NODES, N_EDGES, NUM_BASIS, HIDDEN = 50000, 800000, 8, 256
MUL = 8
INV_SQRT3 = float(1.0 / np.sqrt(3.0))
A_SCALAR = float(np.sqrt(1.0 / 128.0))
A_VECTOR = float(np.sqrt(3.0 / 128.0))
SQRT2 = float(np.sqrt(2.0))
DEG_SCALE = float(1.0 / np.sqrt(N_EDGES / N_NODES))

NCORES = 8
P = 128
NODES_PER_CORE = 6400          # 50 blocks of 128; 8*6400 = 51200 >= 50000
NB = 50                        # node blocks per core
# table: rows 1..50000 = nodes 0..49999; row 50001 = zeros (dummy target).
# gather base = row 32768, int16 idx = node - 32767 in [-32767, 17232];
# dummy idx = +17233 (always non-negative so it never hits the trailing-
# negative trim). Each gather's last (trim-order) index is forced >= 0 by an
# in-block edge swap on the host.
TBL_ROWS = 50004
GBASE = 32768
DUMMY_IDX = 50001 - GBASE

_PROG_CACHE = {}


# ---------------- device program ----------------
def _build_program(NG):
    GROUPS = NB * NG
    WINDOWS = GROUPS // 4            # 4 groups (512 edges) per window
    SUPER_G = 4                      # one gather per window (512 idx <= 1024 cap)
    NSUPER = GROUPS // SUPER_G
    IDXW = SUPER_G * 128 // 16       # wrapped idx cols per super
    NJ = (WINDOWS + 3) // 4          # es_w4 column blocks
    ES_CHUNK_J = 3                   # es col-blocks per streamed chunk
    NCHUNK = (NJ + ES_CHUNK_J - 1) // ES_CHUNK_J

    nc = bacc.Bacc(num_devices=NCORES, num_swdge_queues=4)
    f32, i16 = mybir.dt.float32, mybir.dt.int16

    tbl = nc.dram_tensor("tbl", [TBL_ROWS, 64], f32, kind="ExternalInput")
    idx_g = nc.dram_tensor("idx_g", [P, NSUPER * IDXW], i16, kind="ExternalInput")
    es4 = nc.dram_tensor("es4", [P, NJ * 512], f32, kind="ExternalInput")
    sh_t = nc.dram_tensor("sh_t", [P, GROUPS * 4], f32, kind="ExternalInput")
    dstl = nc.dram_tensor("dstl", [P, GROUPS], f32, kind="ExternalInput")
    w1t = nc.dram_tensor("w1t", [P, 256], f32, kind="ExternalInput")
    w2t = nc.dram_tensor("w2t", [P, 512], f32, kind="ExternalInput")
    iota = nc.dram_tensor("iota", [P, P], f32, kind="ExternalInput")
    nodeout = nc.dram_tensor("nodeout", [NODES_PER_CORE, 32], f32, kind="ExternalOutput")

    AX = mybir.AxisListType.X
    ADD = mybir.AluOpType.add
    MUL_ = mybir.AluOpType.mult
    EQ = mybir.AluOpType.is_equal
    RELU = mybir.ActivationFunctionType.Relu

    with TileContext(nc, trace_sim=_TRACE_SIM) as tc:
        with tc.tile_pool(name="const", bufs=1) as cpool, \
             tc.tile_pool(name="stream", bufs=2) as spool, \
             tc.tile_pool(name="work", bufs=2) as wpool, \
             tc.tile_pool(name="oh", bufs=3) as ohpool, \
             tc.tile_pool(name="psum", bufs=2, space="PSUM") as pp, \
             tc.tile_pool(name="psum1", bufs=1, space="PSUM") as pp1:

            # constants resident in SBUF
            ig_sb = cpool.tile([P, NSUPER * IDXW], i16, name="ig")
            nc.sync.dma_start(ig_sb[:], idx_g[:])
            sh_sb = cpool.tile([P, GROUPS, 4], f32, name="sh")
            nc.sync.dma_start(sh_sb[:], sh_t[:].rearrange("p (g k) -> p g k", k=4))
            dl_sb = cpool.tile([P, GROUPS], f32, name="dl")
            nc.sync.dma_start(dl_sb[:], dstl[:])
            w1_sb = cpool.tile([P, 256], f32, name="w1")
            nc.sync.dma_start(w1_sb[:], w1t[:])
            w2_sb = cpool.tile([P, 2, 256], f32, name="w2")
            nc.sync.dma_start(w2_sb[:], w2t[:].rearrange("p (h n) -> p h n", h=2))
            io_sb = cpool.tile([P, P], f32, name="iota")
            nc.sync.dma_start(io_sb[:], iota[:])

            acc_ps = None
            for w in range(WINDOWS):
                c = w % 4
                j = w // 4
                g0 = 4 * w

                # --- stream es chunk (every ES_CHUNK_J col-blocks)
                if j % ES_CHUNK_J == 0:
                    jw = min(ES_CHUNK_J, NJ - j)
                    es_sb = spool.tile([P, ES_CHUNK_J * 512], f32, tag="es")
                    nc.sync.dma_start(
                        es_sb[:, : jw * 512], es4[:, j * 512 : (j + jw) * 512]
                    )
                jj = j % ES_CHUNK_J

                # --- gather: one 512-idx call per window, cycling queues
                x_c = spool.tile([P, 4, 64], f32, tag="xc")
                if _NO_GATHER:
                    nc.vector.memset(x_c[:, :, 0:4], 0.0)
                elif True:
                    nc.gpsimd.dma_gather(
                    out_ap=x_c[:], in_ap=tbl[GBASE:, :],
                    idxs_ap=ig_sb[:, w * IDXW : (w + 1) * IDXW],
                    num_idxs=512, num_idxs_reg=512, elem_size=64,
                    queue_num=w % 4,
                )
                # --- MLP1: h[comp, edge] for 512 edges, two 128-comp halves
                h_ps = pp1.tile([P, 2, 512], f32, space="PSUM", tag="hps")
                for half in range(2):
                    nc.tensor.matmul(
                        h_ps[:, half, :],
                        lhsT=w1_sb[32 * c : 32 * c + 8, half * 128 : half * 128 + 128],
                        rhs=es_sb[32 * c : 32 * c + 8, jj * 512 : jj * 512 + 512],
                        start=True, stop=True,
                        tile_position=(32 * c, 0),
                    )
                h_sb = wpool.tile([P, 2, 512], f32, tag="hsb")
                for half in range(2):
                    nc.scalar.activation(
                        out=h_sb[:, half, :], in_=h_ps[:, half, :], func=RELU
                    )

                # --- MLP2 per group: w[edge, 256] in PSUM
                w_ps = pp.tile([P, 4, 256], f32, space="PSUM", tag="wps")
                for gg in range(4):
                    for half in range(2):
                        nc.tensor.matmul(
                            w_ps[:, gg, :],
                            lhsT=h_sb[:, half, gg * 128 : gg * 128 + 128],
                            rhs=w2_sb[:, half, :],
                            start=(half == 0), stop=(half == 1),
                        )

                # --- TP products (batched over the 4 groups)
                xs = x_c[:]                           # [P, 4, 64]
                shw = sh_sb[:, g0 : g0 + 4, :]        # [P, 4, 4]
                scat = wpool.tile([P, 4, 512], f32, tag="scat")
                ab16 = wpool.tile([P, 4, 16], f32, tag="ab16")
                # a[u] = s1[u] * s2
                nc.vector.tensor_tensor(
                    out=ab16[:, :, 0:8], in0=xs[:, :, 0:8],
                    in1=shw[:, :, 0:1].to_broadcast([P, 4, 8]), op=MUL_,
                )
                # b[u] = sum_i v1[u,i] * v2[i]
                pb = wpool.tile([P, 4, 8, 3], f32, tag="pb")
                nc.vector.tensor_tensor(
                    out=pb[:],
                    in0=xs[:, :, 8:32].rearrange("p g (u i) -> p g u i", u=8),
                    in1=shw[:, :, 1:4].unsqueeze(2).to_broadcast([P, 4, 8, 3]),
                    op=MUL_,
                )
                nc.vector.tensor_reduce(
                    out=ab16[:, :, 8:16], in_=pb[:], axis=AX, op=ADD
                )
                # ps = ab16[u'] * w01[u', w_]  -> scat[:, :, 0:128]
                nc.vector.tensor_tensor(
                    out=scat[:, :, 0:128].rearrange("p g (u w) -> p g u w", u=16),
                    in0=ab16[:].unsqueeze(3).to_broadcast([P, 4, 16, 8]),
                    in1=w_ps[:, :, 0:128].rearrange("p g (u w) -> p g u w", u=16),
                    op=MUL_,
                )
                # s1v2[u,i] = s1[u]*v2[i]
                s1v2 = wpool.tile([P, 4, 24], f32, tag="s1v2")
                nc.vector.tensor_tensor(
                    out=s1v2[:].rearrange("p g (u i) -> p g u i", u=8),
                    in0=xs[:, :, 0:8].unsqueeze(3).to_broadcast([P, 4, 8, 3]),
                    in1=shw[:, :, 1:4].unsqueeze(2).to_broadcast([P, 4, 8, 3]),
                    op=MUL_,
                )
                # ptv = s1v2[u,i] * w2[u,w_] -> scat cols 128:320 ((u,w_,i))
                nc.vector.tensor_tensor(
                    out=scat[:, :, 128:320].rearrange(
                        "p g (u w i) -> p g u w i", u=8, w=8
                    ),
                    in0=s1v2[:].rearrange("p g (u i) -> p g u i", u=8)
                        .unsqueeze(3).to_broadcast([P, 4, 8, 8, 3]),
                    in1=w_ps[:, :, 128:192]
                        .rearrange("p g (u w) -> p g u w", u=8)
                        .unsqueeze(4).to_broadcast([P, 4, 8, 8, 3]),
                    op=MUL_,
                )
                # v1s2[u,i] = v1[u,i]*s2
                v1s2 = wpool.tile([P, 4, 24], f32, tag="v1s2")
                nc.vector.tensor_tensor(
                    out=v1s2[:], in0=xs[:, :, 8:32],
                    in1=shw[:, :, 0:1].to_broadcast([P, 4, 24]), op=MUL_,
                )
                # pv3 = v1s2[u,i] * w3[u,w_] -> scat cols 320:512
                nc.vector.tensor_tensor(
                    out=scat[:, :, 320:512].rearrange(
                        "p g (u w i) -> p g u w i", u=8, w=8
                    ),
                    in0=v1s2[:].rearrange("p g (u i) -> p g u i", u=8)
                        .unsqueeze(3).to_broadcast([P, 4, 8, 8, 3]),
                    in1=w_ps[:, :, 192:256]
                        .rearrange("p g (u w) -> p g u w", u=8)
                        .unsqueeze(4).to_broadcast([P, 4, 8, 8, 3]),
                    op=MUL_,
                )

                # --- per group: one-hot + scatter matmul into block accumulator
                for gg in range(4):
                    g = g0 + gg
                    b = g // NG
                    gib = g % NG
                    if gib == 0:
                        acc_ps = pp.tile([P, 512], f32, space="PSUM", tag="acc")
                    oh = ohpool.tile([P, P], f32, tag="oh")
                    nc.vector.tensor_scalar(
                        out=oh[:], in0=io_sb[:], scalar1=dl_sb[:, g : g + 1],
                        scalar2=None, op0=EQ,
                    )
                    nc.tensor.matmul(
                        acc_ps[:],
                        lhsT=oh[:], rhs=scat[:, gg, :],
                        start=(gib == 0), stop=(gib == NG - 1),
                    )
                    if gib == NG - 1:
                        stage = wpool.tile([P, 32], f32, tag="stage")
                        nc.vector.tensor_reduce(
                            out=stage[:, 0:8],
                            in_=acc_ps[:, 0:128].rearrange(
                                "p (u w) -> p w u", u=16
                            ),
                            axis=AX, op=ADD,
                        )
                        nc.vector.tensor_reduce(
                            out=stage[:, 8:32],
                            in_=acc_ps[:, 128:512].rearrange(
                                "p (u wi) -> p wi u", u=16
                            ),
                            axis=AX, op=ADD,
                        )
                        nc.sync.dma_start(
                            nodeout[128 * b : 128 * b + 128, :], stage[:]
                        )
    nc.compile()
    return nc


# ---------------- host-side prep ----------------
def _prep(node_features, edge_src, edge_dst, edge_sh, edge_scalars, fc_w1, fc_w2, NG):
    GROUPS = NB * NG
    EPAD = GROUPS * 128
    WINDOWS = GROUPS // 4
    SUPER_G = 4
    NSUPER = GROUPS // SUPER_G
    IDXW = SUPER_G * 128 // 16
    NJ = (WINDOWS + 3) // 4

    # fold all scalar coefficients into the weights
    w1s = (fc_w1 * (1.0 / math.sqrt(NUM_BASIS))).astype(np.float32)     # [8, 256]
    w2 = (fc_w2 * (SQRT2 / math.sqrt(HIDDEN))).astype(np.float64)       # [256, 256]
    w2 = w2.reshape(HIDDEN, 4, MUL, MUL)
    coef = np.array(
        [A_SCALAR, A_SCALAR * INV_SQRT3, A_VECTOR * INV_SQRT3, A_VECTOR * INV_SQRT3]
    ) * DEG_SCALE
    w2 = w2 * coef[None, :, None, None]
    # device col order: [w01 (u'16, w8) | w2 (u8, w8) | w3 (u8, w8)]
    w2dev = np.concatenate(
        [
            w2[:, 0].reshape(HIDDEN, 64),
            w2[:, 1].reshape(HIDDEN, 64),
            w2[:, 2].reshape(HIDDEN, 64),
            w2[:, 3].reshape(HIDDEN, 64),
        ],
        axis=1,
    ).astype(np.float32)                                                # [256, 256]

    w1t = np.zeros((P, 256), np.float32)
    for c in range(4):
        w1t[32 * c : 32 * c + 8] = w1s
    w2t = np.zeros((P, 512), np.float32)
    w2t[:, 0:256] = w2dev[0:128]
    w2t[:, 256:512] = w2dev[128:256]
    iota = np.broadcast_to(np.arange(P, dtype=np.float32), (P, P)).copy()

    tbl = np.zeros((TBL_ROWS, 64), np.float32)
    tbl[1 : N_NODES + 1, 0:32] = node_features

    src_all = np.asarray(edge_src).astype(np.int64)
    dst_all = np.asarray(edge_dst).astype(np.int64)
    es_all = np.asarray(edge_scalars).astype(np.float32)
    sh_all = np.asarray(edge_sh).astype(np.float32)
    core_of = dst_all // NODES_PER_CORE

    in_maps = []
    for cid in range(NCORES):
        sel = np.nonzero(core_of == cid)[0]
        d = dst_all[sel]
        order = np.argsort(d, kind="stable")
        sel = sel[order]
        d = d[order]
        blk = (d - NODES_PER_CORE * cid) >> 7
        cnt = np.bincount(blk, minlength=NB)
        assert cnt.max() <= NG * 128, (cid, cnt.max())
        start = np.zeros(NB, np.int64)
        start[1:] = np.cumsum(cnt)[:-1]
        rank = np.arange(len(sel)) - start[blk]
        slot = blk * (NG * 128) + rank

        srcv = np.full(EPAD, -1, np.int64)
        srcv[slot] = src_all[sel]
        shv = np.zeros((EPAD, 4), np.float32)
        shv[slot] = sh_all[sel]
        esv = np.zeros((EPAD, 8), np.float32)
        esv[slot] = es_all[sel]
        dlv = np.zeros(EPAD, np.float32)
        dlv[slot] = (d - NODES_PER_CORE * cid - 128 * blk).astype(np.float32)

        # --- gather indices: idx = node - (GBASE - 1 - 1)... row = node+1,
        # idx = row - GBASE = node + 1 - GBASE; dummy -> DUMMY_IDX (>= 0)
        idxv = np.where(srcv >= 0, srcv + 1 - GBASE, DUMMY_IDX).astype(np.int64)
        # force the trim-order-last index of each 512-idx gather call to be
        # >= 0 by swapping that edge with a non-negative-idx edge of the SAME
        # node block (any within-block permutation is valid).
        BLKE = NG * 128
        for sgi in range(NSUPER):
            jl = (sgi + 1) * SUPER_G * 128 - 1
            if idxv[jl] >= 0:
                continue
            b0 = jl // BLKE
            cand = np.nonzero(idxv[b0 * BLKE : (b0 + 1) * BLKE] >= 0)[0]
            # exclude other supers' final slots
            cand = [b0 * BLKE + q for q in cand
                    if (b0 * BLKE + q + 1) % (SUPER_G * 128) != 0]
            assert cand, "no swap candidate in block"
            q = cand[0]
            for arr in (idxv, srcv, dlv):
                arr[jl], arr[q] = arr[q], arr[jl]
            for arr in (shv, esv):
                tmpq = arr[q].copy()
                arr[q] = arr[jl]
                arr[jl] = tmpq
        idx_g = np.tile(
            idxv.reshape(-1, 16).T.astype(np.int16), (8, 1)
        )  # wrap is uniform: IDXW*NSUPER cols total

        # es4: window w at rows 32*(w%4)+b, cols [ (w//4)*512, +512 )
        es4 = np.zeros((P, NJ * 512), np.float32)
        esw = esv.reshape(WINDOWS, 512, 8)
        for c in range(4):
            wsel = np.arange(c, WINDOWS, 4)       # these windows use strip c
            nw = len(wsel)                        # w//4 == index within wsel
            es4[32 * c : 32 * c + 8, : nw * 512] = (
                esw[wsel].transpose(2, 0, 1).reshape(8, nw * 512)
            )

        sh_t = shv.reshape(GROUPS, P, 4).transpose(1, 0, 2).reshape(P, GROUPS * 4)
        dstl = dlv.reshape(GROUPS, P).T.copy()

        in_maps.append(
            dict(
                tbl=tbl, idx_g=np.ascontiguousarray(idx_g),
                es4=np.ascontiguousarray(es4),
                sh_t=np.ascontiguousarray(sh_t),
                dstl=np.ascontiguousarray(dstl),
                w1t=w1t, w2t=w2t, iota=iota,
            )
        )
    return in_maps


def _compute_ng(edge_dst):
    dst_all = np.asarray(edge_dst).astype(np.int64)
    gblk = (dst_all // NODES_PER_CORE) * NB + ((dst_all % NODES_PER_CORE) >> 7)
    cnt = np.bincount(gblk, minlength=NB * NCORES)
    NG = int(math.ceil(cnt.max() / 128.0))
    if NG % 2:
        NG += 1
    return max(NG, 2)


def kernel(node_features, edge_src, edge_dst, edge_sh, edge_scalars, fc_w1, fc_w2):
    node_features = np.asarray(node_features, dtype=np.float32)
    edge_sh = np.asarray(edge_sh, dtype=np.float32)
    edge_scalars = np.asarray(edge_scalars, dtype=np.float32)
    fc_w1 = np.asarray(fc_w1, dtype=np.float32)
    fc_w2 = np.asarray(fc_w2, dtype=np.float32)

    NG = _compute_ng(edge_dst)
    if NG not in _PROG_CACHE:
        _PROG_CACHE[NG] = _build_program(NG)
    nc = _PROG_CACHE[NG]

    in_maps = _prep(
        node_features, edge_src, edge_dst, edge_sh, edge_scalars, fc_w1, fc_w2, NG
    )
    res = run_bass_kernel_spmd(nc, in_maps, core_ids=list(range(NCORES)))
    out = np.concatenate([res.results[c]["nodeout"] for c in range(NCORES)], axis=0)
    return out[:N_NODES].astype(np.float32)



# revision 11
# speedup vs baseline: 2.8251x; 2.8251x over previous
"""Trainium2 Bass kernel for nn_Convolution (e3nn-style GNN message passing).

Strategy (8 NeuronCores, SPMD, no collectives):
- Sort edges by destination; core c owns destination nodes [6400c, 6400(c+1)).
- Per core: edges are binned into 50 node-blocks (128 nodes each) and padded to
  NG groups of 128 edges per block. Dummy edges gather a zero table row, so
  every tensor-product output term (all bilinear in source features) is 0.
- Gather source features with dma_gather from a 256B-padded table, split into
  lo/hi halves (int16 index limit), summed on DVE.
- Radial MLP layer 1 on PE with tile_position row-packed K=8 matmuls,
  layer 2 per-group with h as the stationary operand (w lands [edge, 256]).
- TP products on DVE via broadcast APs; the per-edge contraction over u is
  DEFERRED into the scatter matmul: one-hot(dst) x [512-wide product tile]
  accumulates in PSUM over each block, reduced over u once per block.
"""

import math
import os
import ml_dtypes
import numpy as np

BF16 = ml_dtypes.bfloat16

_TRACE_SIM = bool(int(os.environ.get('K_TRACE_SIM', '0')))
_NO_GATHER = bool(int(os.environ.get('K_NO_GATHER', '0')))
_NO_TP = bool(int(os.environ.get('K_NO_TP', '0')))
_NO_MM = bool(int(os.environ.get('K_NO_MM', '0')))


import concourse.bass as bass
import concourse.bacc as bacc
import concourse.mybir as mybir
from concourse.tile import TileContext
from concourse.bass_utils import run_bass_kernel_spmd

# ---------------- problem constants (hardcoded per spec) ----------------
N_NODES, N_EDGES, NUM_BASIS, HIDDEN = 50000, 800000, 8, 256
MUL = 8
INV_SQRT3 = float(1.0 / np.sqrt(3.0))
A_SCALAR = float(np.sqrt(1.0 / 128.0))
A_VECTOR = float(np.sqrt(3.0 / 128.0))
SQRT2 = float(np.sqrt(2.0))
DEG_SCALE = float(1.0 / np.sqrt(N_EDGES / N_NODES))

NCORES = 8
P = 128
NODES_PER_CORE = 6400          # 50 blocks of 128; 8*6400 = 51200 >= 50000
NB = 50                        # node blocks per core
# table: rows 1..50000 = nodes 0..49999; row 50001 = zeros (dummy target).
# gather base = row 32768, int16 idx = node - 32767 in [-32767, 17232];
# dummy idx = +17233 (always non-negative so it never hits the trailing-
# negative trim). Each gather's last (trim-order) index is forced >= 0 by an
# in-block edge swap on the host.
TBL_ROWS = 50004
GBASE = 32768
DUMMY_IDX = 50001 - GBASE

_PROG_CACHE = {}


# ---------------- device program ----------------
def _build_program(NG):
    GROUPS = NB * NG
    WINDOWS = GROUPS // 4            # 4 groups (512 edges) per window
    SUPER_G = 4                      # one gather per window (512 idx <= 1024 cap)
    NSUPER = GROUPS // SUPER_G
    IDXW = SUPER_G * 128 // 16       # wrapped idx cols per super
    NJ = (WINDOWS + 3) // 4          # es_w4 column blocks
    ES_CHUNK_J = 3                   # es col-blocks per streamed chunk
    NCHUNK = (NJ + ES_CHUNK_J - 1) // ES_CHUNK_J

    nc = bacc.Bacc(num_devices=NCORES, num_swdge_queues=4)
    f32, i16 = mybir.dt.float32, mybir.dt.int16
    bf16 = mybir.dt.bfloat16

    tbl = nc.dram_tensor("tbl", [TBL_ROWS, 64], f32, kind="ExternalInput")
    idx_g = nc.dram_tensor("idx_g", [P, NSUPER * IDXW], i16, kind="ExternalInput")
    es4 = nc.dram_tensor("es4", [P, NJ * 512], bf16, kind="ExternalInput")
    sh_t = nc.dram_tensor("sh_t", [P, GROUPS * 4], f32, kind="ExternalInput")
    dstl = nc.dram_tensor("dstl", [P, GROUPS], f32, kind="ExternalInput")
    w1t = nc.dram_tensor("w1t", [P, 256], bf16, kind="ExternalInput")
    w2t = nc.dram_tensor("w2t", [P, 512], bf16, kind="ExternalInput")
    iota = nc.dram_tensor("iota", [P, P], bf16, kind="ExternalInput")
    nodeout = nc.dram_tensor("nodeout", [NODES_PER_CORE, 32], f32, kind="ExternalOutput")

    AX = mybir.AxisListType.X
    ADD = mybir.AluOpType.add
    MUL_ = mybir.AluOpType.mult
    EQ = mybir.AluOpType.is_equal
    RELU = mybir.ActivationFunctionType.Relu

    with TileContext(nc, trace_sim=_TRACE_SIM) as tc:
        with tc.tile_pool(name="const", bufs=1) as cpool, \
             tc.tile_pool(name="stream", bufs=2) as spool, \
             tc.tile_pool(name="work", bufs=2) as wpool, \
             tc.tile_pool(name="oh", bufs=3) as ohpool, \
             tc.tile_pool(name="psum", bufs=2, space="PSUM") as pp, \
             tc.tile_pool(name="psum1", bufs=1, space="PSUM") as pp1:

            # constants resident in SBUF
            ig_sb = cpool.tile([P, NSUPER * IDXW], i16, name="ig")
            nc.sync.dma_start(ig_sb[:], idx_g[:])
            sh_sb = cpool.tile([P, GROUPS, 4], f32, name="sh")
            nc.sync.dma_start(sh_sb[:], sh_t[:].rearrange("p (g k) -> p g k", k=4))
            dl_sb = cpool.tile([P, GROUPS], f32, name="dl")
            nc.sync.dma_start(dl_sb[:], dstl[:])
            w1_sb = cpool.tile([P, 256], bf16, name="w1")
            nc.sync.dma_start(w1_sb[:], w1t[:])
            w2_sb = cpool.tile([P, 2, 256], bf16, name="w2")
            nc.sync.dma_start(w2_sb[:], w2t[:].rearrange("p (h n) -> p h n", h=2))
            io_sb = cpool.tile([P, P], bf16, name="iota")
            nc.sync.dma_start(io_sb[:], iota[:])

            acc_ps = None
            for w in range(WINDOWS):
                c = w % 4
                j = w // 4
                g0 = 4 * w

                # --- stream es chunk (every ES_CHUNK_J col-blocks)
                if j % ES_CHUNK_J == 0:
                    jw = min(ES_CHUNK_J, NJ - j)
                    es_sb = spool.tile([P, ES_CHUNK_J * 512], bf16, tag="es")
                    nc.sync.dma_start(
                        es_sb[:, : jw * 512], es4[:, j * 512 : (j + jw) * 512]
                    )
                jj = j % ES_CHUNK_J

                # --- gather: one 512-idx call per window, cycling queues
                x_c = spool.tile([P, 4, 64], f32, tag="xc")
                if _NO_GATHER:
                    nc.vector.memset(x_c[:, :, 0:4], 0.0)
                elif True:
                    nc.gpsimd.dma_gather(
                    out_ap=x_c[:], in_ap=tbl[GBASE:, :],
                    idxs_ap=ig_sb[:, w * IDXW : (w + 1) * IDXW],
                    num_idxs=512, num_idxs_reg=512, elem_size=64,
                    queue_num=w % 4,
                )
                # --- MLP1: h[comp, edge] for 512 edges, two 128-comp halves
                h_ps = pp1.tile([P, 2, 512], f32, space="PSUM", tag="hps")
                for half in range(2):
                    nc.tensor.matmul(
                        h_ps[:, half, :],
                        lhsT=w1_sb[32 * c : 32 * c + 8, half * 128 : half * 128 + 128],
                        rhs=es_sb[32 * c : 32 * c + 8, jj * 512 : jj * 512 + 512],
                        start=True, stop=True,
                        tile_position=(32 * c, 0),
                    )
                h_sb = wpool.tile([P, 2, 512], bf16, tag="hsb")
                for half in range(2):
                    nc.scalar.activation(
                        out=h_sb[:, half, :], in_=h_ps[:, half, :], func=RELU
                    )

                # --- MLP2 per group: w[edge, 256] in PSUM
                w_ps = pp.tile([P, 4, 256], f32, space="PSUM", tag="wps")
                for gg in range(4):
                    for half in range(2):
                        nc.tensor.matmul(
                            w_ps[:, gg, :],
                            lhsT=h_sb[:, half, gg * 128 : gg * 128 + 128],
                            rhs=w2_sb[:, half, :],
                            start=(half == 0), stop=(half == 1),
                        )

                # --- TP products (batched over the 4 groups)
                xs = x_c[:]                           # [P, 4, 64]
                shw = sh_sb[:, g0 : g0 + 4, :]        # [P, 4, 4]
                scat = wpool.tile([P, 4, 512], bf16, tag="scat")
                ab16 = wpool.tile([P, 4, 16], f32, tag="ab16")
                # a[u] = s1[u] * s2
                nc.vector.tensor_tensor(
                    out=ab16[:, :, 0:8], in0=xs[:, :, 0:8],
                    in1=shw[:, :, 0:1].to_broadcast([P, 4, 8]), op=MUL_,
                )
                # b[u] = sum_i v1[u,i] * v2[i]
                pb = wpool.tile([P, 4, 8, 3], f32, tag="pb")
                nc.vector.tensor_tensor(
                    out=pb[:],
                    in0=xs[:, :, 8:32].rearrange("p g (u i) -> p g u i", u=8),
                    in1=shw[:, :, 1:4].unsqueeze(2).to_broadcast([P, 4, 8, 3]),
                    op=MUL_,
                )
                nc.vector.tensor_reduce(
                    out=ab16[:, :, 8:16], in_=pb[:], axis=AX, op=ADD
                )
                # ps = ab16[u'] * w01[u', w_]  -> scat[:, :, 0:128]
                nc.vector.tensor_tensor(
                    out=scat[:, :, 0:128].rearrange("p g (u w) -> p g u w", u=16),
                    in0=ab16[:].unsqueeze(3).to_broadcast([P, 4, 16, 8]),
                    in1=w_ps[:, :, 0:128].rearrange("p g (u w) -> p g u w", u=16),
                    op=MUL_,
                )
                # s1v2[u,i] = s1[u]*v2[i]
                s1v2 = wpool.tile([P, 4, 24], f32, tag="s1v2")
                nc.vector.tensor_tensor(
                    out=s1v2[:].rearrange("p g (u i) -> p g u i", u=8),
                    in0=xs[:, :, 0:8].unsqueeze(3).to_broadcast([P, 4, 8, 3]),
                    in1=shw[:, :, 1:4].unsqueeze(2).to_broadcast([P, 4, 8, 3]),
                    op=MUL_,
                )
                # ptv = s1v2[u,i] * w2[u,w_] -> scat cols 128:320 ((u,w_,i))
                nc.vector.tensor_tensor(
                    out=scat[:, :, 128:320].rearrange(
                        "p g (u w i) -> p g u w i", u=8, w=8
                    ),
                    in0=s1v2[:].rearrange("p g (u i) -> p g u i", u=8)
                        .unsqueeze(3).to_broadcast([P, 4, 8, 8, 3]),
                    in1=w_ps[:, :, 128:192]
                        .rearrange("p g (u w) -> p g u w", u=8)
                        .unsqueeze(4).to_broadcast([P, 4, 8, 8, 3]),
                    op=MUL_,
                )
                # v1s2[u,i] = v1[u,i]*s2
                v1s2 = wpool.tile([P, 4, 24], f32, tag="v1s2")
                nc.vector.tensor_tensor(
                    out=v1s2[:], in0=xs[:, :, 8:32],
                    in1=shw[:, :, 0:1].to_broadcast([P, 4, 24]), op=MUL_,
                )
                # pv3 = v1s2[u,i] * w3[u,w_] -> scat cols 320:512
                nc.vector.tensor_tensor(
                    out=scat[:, :, 320:512].rearrange(
                        "p g (u w i) -> p g u w i", u=8, w=8
                    ),
                    in0=v1s2[:].rearrange("p g (u i) -> p g u i", u=8)
                        .unsqueeze(3).to_broadcast([P, 4, 8, 8, 3]),
                    in1=w_ps[:, :, 192:256]
                        .rearrange("p g (u w) -> p g u w", u=8)
                        .unsqueeze(4).to_broadcast([P, 4, 8, 8, 3]),
                    op=MUL_,
                )

                # --- per group: one-hot + scatter matmul into block accumulator
                for gg in range(4):
                    g = g0 + gg
                    b = g // NG
                    gib = g % NG
                    if gib == 0:
                        acc_ps = pp.tile([P, 512], f32, space="PSUM", tag="acc")
                    oh = ohpool.tile([P, P], bf16, tag="oh")
                    nc.vector.tensor_scalar(
                        out=oh[:], in0=io_sb[:], scalar1=dl_sb[:, g : g + 1],
                        scalar2=None, op0=EQ,
                    )
                    nc.tensor.matmul(
                        acc_ps[:],
                        lhsT=oh[:], rhs=scat[:, gg, :],
                        start=(gib == 0), stop=(gib == NG - 1),
                    )
                    if gib == NG - 1:
                        stage = wpool.tile([P, 32], f32, tag="stage")
                        nc.vector.tensor_reduce(
                            out=stage[:, 0:8],
                            in_=acc_ps[:, 0:128].rearrange(
                                "p (u w) -> p w u", u=16
                            ),
                            axis=AX, op=ADD,
                        )
                        nc.vector.tensor_reduce(
                            out=stage[:, 8:32],
                            in_=acc_ps[:, 128:512].rearrange(
                                "p (u wi) -> p wi u", u=16
                            ),
                            axis=AX, op=ADD,
                        )
                        nc.sync.dma_start(
                            nodeout[128 * b : 128 * b + 128, :], stage[:]
                        )
    nc.compile()
    return nc


# ---------------- host-side prep ----------------
def _prep(node_features, edge_src, edge_dst, edge_sh, edge_scalars, fc_w1, fc_w2, NG):
    GROUPS = NB * NG
    EPAD = GROUPS * 128
    WINDOWS = GROUPS // 4
    SUPER_G = 4
    NSUPER = GROUPS // SUPER_G
    IDXW = SUPER_G * 128 // 16
    NJ = (WINDOWS + 3) // 4

    # fold all scalar coefficients into the weights
    w1s = (fc_w1 * (1.0 / math.sqrt(NUM_BASIS))).astype(np.float32)     # [8, 256]
    w2 = (fc_w2 * (SQRT2 / math.sqrt(HIDDEN))).astype(np.float64)       # [256, 256]
    w2 = w2.reshape(HIDDEN, 4, MUL, MUL)
    coef = np.array(
        [A_SCALAR, A_SCALAR * INV_SQRT3, A_VECTOR * INV_SQRT3, A_VECTOR * INV_SQRT3]
    ) * DEG_SCALE
    w2 = w2 * coef[None, :, None, None]
    # device col order: [w01 (u'16, w8) | w2 (u8, w8) | w3 (u8, w8)]
    w2dev = np.concatenate(
        [
            w2[:, 0].reshape(HIDDEN, 64),
            w2[:, 1].reshape(HIDDEN, 64),
            w2[:, 2].reshape(HIDDEN, 64),
            w2[:, 3].reshape(HIDDEN, 64),
        ],
        axis=1,
    ).astype(np.float32)                                                # [256, 256]

    w1t = np.zeros((P, 256), BF16)
    for c in range(4):
        w1t[32 * c : 32 * c + 8] = w1s.astype(BF16)
    w2t = np.zeros((P, 512), BF16)
    w2t[:, 0:256] = w2dev[0:128].astype(BF16)
    w2t[:, 256:512] = w2dev[128:256].astype(BF16)
    iota = np.broadcast_to(
        np.arange(P, dtype=np.float32).astype(BF16), (P, P)
    ).copy()

    tbl = np.zeros((TBL_ROWS, 64), np.float32)
    tbl[1 : N_NODES + 1, 0:32] = node_features

    src_all = np.asarray(edge_src).astype(np.int64)
    dst_all = np.asarray(edge_dst).astype(np.int64)
    es_all = np.asarray(edge_scalars).astype(np.float32)
    sh_all = np.asarray(edge_sh).astype(np.float32)
    core_of = dst_all // NODES_PER_CORE

    in_maps = []
    for cid in range(NCORES):
        sel = np.nonzero(core_of == cid)[0]
        d = dst_all[sel]
        order = np.argsort(d, kind="stable")
        sel = sel[order]
        d = d[order]
        blk = (d - NODES_PER_CORE * cid) >> 7
        cnt = np.bincount(blk, minlength=NB)
        assert cnt.max() <= NG * 128, (cid, cnt.max())
        start = np.zeros(NB, np.int64)
        start[1:] = np.cumsum(cnt)[:-1]
        rank = np.arange(len(sel)) - start[blk]
        slot = blk * (NG * 128) + rank

        srcv = np.full(EPAD, -1, np.int64)
        srcv[slot] = src_all[sel]
        shv = np.zeros((EPAD, 4), np.float32)
        shv[slot] = sh_all[sel]
        esv = np.zeros((EPAD, 8), np.float32)
        esv[slot] = es_all[sel]
        dlv = np.zeros(EPAD, np.float32)
        dlv[slot] = (d - NODES_PER_CORE * cid - 128 * blk).astype(np.float32)

        # --- gather indices: idx = node - (GBASE - 1 - 1)... row = node+1,
        # idx = row - GBASE = node + 1 - GBASE; dummy -> DUMMY_IDX (>= 0)
        idxv = np.where(srcv >= 0, srcv + 1 - GBASE, DUMMY_IDX).astype(np.int64)
        # force the trim-order-last index of each 512-idx gather call to be
        # >= 0 by swapping that edge with a non-negative-idx edge of the SAME
        # node block (any within-block permutation is valid).
        BLKE = NG * 128
        for sgi in range(NSUPER):
            jl = (sgi + 1) * SUPER_G * 128 - 1
            if idxv[jl] >= 0:
                continue
            b0 = jl // BLKE
            cand = np.nonzero(idxv[b0 * BLKE : (b0 + 1) * BLKE] >= 0)[0]
            # exclude other supers' final slots
            cand = [b0 * BLKE + q for q in cand
                    if (b0 * BLKE + q + 1) % (SUPER_G * 128) != 0]
            assert cand, "no swap candidate in block"
            q = cand[0]
            for arr in (idxv, srcv, dlv):
                arr[jl], arr[q] = arr[q], arr[jl]
            for arr in (shv, esv):
                tmpq = arr[q].copy()
                arr[q] = arr[jl]
                arr[jl] = tmpq
        idx_g = np.tile(
            idxv.reshape(-1, 16).T.astype(np.int16), (8, 1)
        )  # wrap is uniform: IDXW*NSUPER cols total

        # es4: window w at rows 32*(w%4)+b, cols [ (w//4)*512, +512 )
        es4 = np.zeros((P, NJ * 512), BF16)
        esw = esv.reshape(WINDOWS, 512, 8)
        for c in range(4):
            wsel = np.arange(c, WINDOWS, 4)       # these windows use strip c
            nw = len(wsel)                        # w//4 == index within wsel
            es4[32 * c : 32 * c + 8, : nw * 512] = (
                esw[wsel].transpose(2, 0, 1).reshape(8, nw * 512)
            )

        sh_t = shv.reshape(GROUPS, P, 4).transpose(1, 0, 2).reshape(P, GROUPS * 4)
        dstl = dlv.reshape(GROUPS, P).T.copy()

        in_maps.append(
            dict(
                tbl=tbl, idx_g=np.ascontiguousarray(idx_g),
                es4=np.ascontiguousarray(es4),
                sh_t=np.ascontiguousarray(sh_t),
                dstl=np.ascontiguousarray(dstl),
                w1t=w1t, w2t=w2t, iota=iota,
            )
        )
    return in_maps


def _compute_ng(edge_dst):
    dst_all = np.asarray(edge_dst).astype(np.int64)
    gblk = (dst_all // NODES_PER_CORE) * NB + ((dst_all % NODES_PER_CORE) >> 7)
    cnt = np.bincount(gblk, minlength=NB * NCORES)
    NG = int(math.ceil(cnt.max() / 128.0))
    if NG % 2:
        NG += 1
    return max(NG, 2)


def kernel(node_features, edge_src, edge_dst, edge_sh, edge_scalars, fc_w1, fc_w2):
    node_features = np.asarray(node_features, dtype=np.float32)
    edge_sh = np.asarray(edge_sh, dtype=np.float32)
    edge_scalars = np.asarray(edge_scalars, dtype=np.float32)
    fc_w1 = np.asarray(fc_w1, dtype=np.float32)
    fc_w2 = np.asarray(fc_w2, dtype=np.float32)

    NG = _compute_ng(edge_dst)
    if NG not in _PROG_CACHE:
        _PROG_CACHE[NG] = _build_program(NG)
    nc = _PROG_CACHE[NG]

    in_maps = _prep(
        node_features, edge_src, edge_dst, edge_sh, edge_scalars, fc_w1, fc_w2, NG
    )
    res = run_bass_kernel_spmd(nc, in_maps, core_ids=list(range(NCORES)))
    out = np.concatenate([res.results[c]["nodeout"] for c in range(NCORES)], axis=0)
    return out[:N_NODES].astype(np.float32)

